# revision 1
# baseline (speedup 1.0000x reference)
"""DynamicCylinderFeatureNet on 8 Trainium2 NeuronCores (Bass/Tile).

Distribution strategy
---------------------
Voxel-range sharding: voxels are split into 8 contiguous id-ranges with
~equal point counts, and every point goes to the core that owns its voxel.
All points of a voxel therefore live on one core, so each segment_max is
fully core-local and no collectives are needed.

segment_max without gathers
---------------------------
On the host, each core's points are reordered into "count-class chunks". A
chunk (c, G) holds G voxels that each have exactly c points (voxel counts are
rounded up into a small class set by duplicating points - duplicates never
change a max). Inside a chunk, point column j*G + g is copy j of voxel g.
Then:
  * segment_max  = c-1 dense elementwise maxes over contiguous slices
  * vf[inv] gather-back = plain slice reuse (same voxel slice for every j)
so the device program is completely dense: strided DMAs, matmuls, per-channel
activations and elementwise maxes. The only index work happens on the host
(pure integer planning + input permutation; all FLOPs run on device).

Device program (per core, per chunk; layout [channels, points] on SBUF):
  h0 = W0f.T @ feat9 - W0[3:6].T @ centers_bcast   (K-split matmul does the
       per-point "x - voxel_center" concat for free; centers computed on
       device from int voxel coords)
  p0 = relu(a0*h0 + b0)  (BN folded on host into per-channel a, b)
  vf0 = max_j p0-slices ; layer1/2 analogous with K-split matmuls reading
  [p, vf] directly from separate SBUF tiles. Layer-2 seg-max runs on raw
  matmul outputs (valid: a2 > 0), activation applied per-voxel, projection
  fused on-chip. Matmuls run as float32r (full PE rate, fp32 storage).
"""
import sys
import numpy as np

if "/opt/trn_rl_repo" not in sys.path:
    sys.path.insert(0, "/opt/trn_rl_repo")

N_CORES = 8
EPS = 1e-3
_CYL_RANGE = np.array([-4.0, -np.pi, 0.0, 2.0, np.pi, 50.0], np.float32)
_PART = np.array([32.0, 360.0, 480.0], np.float32)
VOXEL_SIZE = ((_CYL_RANGE[3:] - _CYL_RANGE[:3]) / _PART).astype(np.float32)
PC_MIN = _CYL_RANGE[:3]

PC_MAX = 3584      # max points per chunk
G_MAX = 1792       # max voxels per chunk
TILE_N = 512       # matmul free-dim tile


# --------------------------------------------------------------------------
# host-side planning
# --------------------------------------------------------------------------

def _class_set(cmax):
    cls = list(range(1, 9))
    c = 10
    while cls[-1] < cmax:
        cls.append(c)
        c += 2 if c < 16 else (4 if c < 32 else 8)
    return cls


def build_plan(inv, V):
    N = inv.shape[0]
    counts = np.bincount(inv, minlength=V)
    assert counts.min() >= 1, "every voxel must have at least one point"
    cum = np.cumsum(counts)
    bounds = [0]
    for k in range(1, N_CORES):
        bounds.append(int(np.searchsorted(cum, N * k // N_CORES)))
    bounds.append(V)

    cls = _class_set(int(counts.max()))
    cls_arr = np.array(cls)

    order = np.argsort(inv, kind="stable")
    starts = np.zeros(V + 1, np.int64)
    starts[1:] = cum

    core_cls_vox = []
    Gd = np.zeros((N_CORES, len(cls)), np.int64)
    for d in range(N_CORES):
        vlo, vhi = bounds[d], bounds[d + 1]
        vcls_idx = np.searchsorted(cls_arr, counts[vlo:vhi])
        lists = [np.arange(vlo, vhi)[vcls_idx == i] for i in range(len(cls))]
        core_cls_vox.append(lists)
        Gd[d] = [len(x) for x in lists]
    Gstar = Gd.max(axis=0)

    chunks = []
    cls_chunks = []
    for c, Gs in zip(cls, Gstar):
        if Gs == 0:
            cls_chunks.append([])
            continue
        gmax = max(1, min(G_MAX, PC_MAX // c))
        sizes = []
        g = 0
        while g < Gs:
            s = min(gmax, Gs - g)
            sizes.append(s)
            g += s
        cls_chunks.append(sizes)
        for s in sizes:
            chunks.append((c, s))

    Vd_pad = int(sum(G for _, G in chunks))
    P_pad = int(sum(c * G for c, G in chunks))

    per_core = []
    for d in range(N_CORES):
        vox_ids = np.full(Vd_pad, -1, np.int64)
        point_src = np.full(P_pad, -1, np.int64)
        gbase = 0
        pbase = 0
        for ci, (c, sizes) in enumerate(zip(cls, cls_chunks)):
            sel = core_cls_vox[d][ci]
            off = 0
            for G in sizes:
                sub = sel[off:off + G]
                m = len(sub)
                vox_ids[gbase:gbase + m] = sub
                if m:
                    J = np.arange(c)[:, None]
                    idx = starts[sub][None, :] + (J % counts[sub][None, :])
                    slots = pbase + J * G + np.arange(m)[None, :]
                    point_src[slots.ravel()] = order[idx.ravel()]
                off += G
                gbase += G
                pbase += c * G
        per_core.append(dict(vox_ids=vox_ids, point_src=point_src))
    return dict(chunks=chunks, Vd_pad=Vd_pad, P_pad=P_pad, per_core=per_core)


def _fold_bn(g, b, m, v):
    a = np.asarray(g, np.float32) / np.sqrt(np.asarray(v, np.float32) + EPS)
    return (a.astype(np.float32),
            (np.asarray(b, np.float32) - np.asarray(m, np.float32) * a).astype(np.float32))


def host_prepare(plan, inputs):
    feats = np.asarray(inputs["features"], np.float32)
    coors = np.asarray(inputs["voxel_coors"])
    a0, b0 = _fold_bn(inputs["g0"], inputs["b0"], inputs["m0"], inputs["v0"])
    a1, b1 = _fold_bn(inputs["g1"], inputs["b1"], inputs["m1"], inputs["v1"])
    a2, b2 = _fold_bn(inputs["g2"], inputs["b2"], inputs["m2"], inputs["v2"])
    assert (a2 > 0).all(), "layer-2 pre-activation segmax needs a2 > 0"
    W0 = np.ascontiguousarray(np.asarray(inputs["W0"], np.float32))
    W1 = np.asarray(inputs["W1"], np.float32)
    W2 = np.asarray(inputs["W2"], np.float32)
    Wp = np.asarray(inputs["Wp"], np.float32)
    csb = np.stack([VOXEL_SIZE, 0.5 * VOXEL_SIZE + PC_MIN], axis=1).astype(np.float32)
    shared = {
        "W0f": W0,
        "W0c": np.ascontiguousarray(-W0[3:6]),
        "W1a": np.ascontiguousarray(W1[:64]),
        "W1b": np.ascontiguousarray(W1[64:]),
        "W2aa": np.ascontiguousarray(W2[:128, :128]),
        "W2ba": np.ascontiguousarray(W2[128:, :128]),
        "W2ab": np.ascontiguousarray(W2[:128, 128:]),
        "W2bb": np.ascontiguousarray(W2[128:, 128:]),
        "Wpa": np.ascontiguousarray(Wp[:128]),
        "Wpb": np.ascontiguousarray(Wp[128:]),
        "ab0": np.stack([a0, b0], 1),
        "ab1": np.stack([a1, b1], 1),
        "ab2a": np.stack([a2[:128], b2[:128]], 1),
        "ab2b": np.stack([a2[128:], b2[128:]], 1),
        "bp": np.asarray(inputs["bp"], np.float32).reshape(16, 1),
        "csb": csb,
    }
    in_maps = []
    for d in range(N_CORES):
        pc = plan["per_core"][d]
        src = pc["point_src"]
        ok = src >= 0
        feat9 = np.zeros((9, plan["P_pad"]), np.float32)
        feat9[0:3, ok] = feats[src[ok], 0:3].T
        feat9[3:6, ok] = feats[src[ok], 0:3].T
        feat9[6:9, ok] = feats[src[ok], 3:6].T
        vids = pc["vox_ids"]
        vok = vids >= 0
        coorsT = np.zeros((3, plan["Vd_pad"]), np.int32)
        coorsT[:, vok] = coors[vids[vok], 1:4].astype(np.int32).T
        m = dict(shared)
        m["feat9"] = feat9
        m["coorsT"] = coorsT
        in_maps.append(m)
    return in_maps


def host_assemble(plan, core_outs, V):
    out = np.zeros((V, 16), np.float32)
    for d in range(N_CORES):
        vids = plan["per_core"][d]["vox_ids"]
        vok = vids >= 0
        out[vids[vok]] = core_outs[d][:, vok].T
    return out


# --------------------------------------------------------------------------
# device program
# --------------------------------------------------------------------------

def build_program(plan):
    import concourse.mybir as mybir
    import concourse.tile as tile
    from concourse import bacc

    f32 = mybir.dt.float32
    f32r = mybir.dt.float32r
    i32 = mybir.dt.int32
    AF = mybir.ActivationFunctionType
    Alu = mybir.AluOpType

    P_pad, Vd_pad = plan["P_pad"], plan["Vd_pad"]
    chunks = plan["chunks"]

    nc = bacc.Bacc(None, target_bir_lowering=False, debug=False)

    feat9 = nc.dram_tensor("feat9", [9, P_pad], f32, kind="ExternalInput")
    coorsT = nc.dram_tensor("coorsT", [3, Vd_pad], i32, kind="ExternalInput")
    wnames = {
        "W0f": [9, 64], "W0c": [3, 64],
        "W1a": [64, 128], "W1b": [64, 128],
        "W2aa": [128, 128], "W2ba": [128, 128],
        "W2ab": [128, 128], "W2bb": [128, 128],
        "Wpa": [128, 16], "Wpb": [128, 16],
        "ab0": [64, 2], "ab1": [128, 2],
        "ab2a": [128, 2], "ab2b": [128, 2],
        "bp": [16, 1], "csb": [3, 2],
    }
    wdram = {k: nc.dram_tensor(k, s, f32, kind="ExternalInput")
             for k, s in wnames.items()}
    out_T = nc.dram_tensor("out_T", [16, Vd_pad], f32, kind="ExternalOutput")

    def r(ap):
        return ap.bitcast(f32r)

    with tile.TileContext(nc) as tc:
        with tc.tile_pool(name="const", bufs=1) as cpool, \
             tc.tile_pool(name="work", bufs=2) as pool, \
             tc.tile_pool(name="psum", bufs=2, space="PSUM") as pp:
            wt = {}
            for k, s in wnames.items():
                wt[k] = cpool.tile(s, f32, tag=k, name=k)
                nc.sync.dma_start(out=wt[k][:], in_=wdram[k][:])

            pbase = 0
            gbase = 0
            for (c, G) in chunks:
                Pc = c * G
                ft = pool.tile([9, Pc], f32, tag="feat9", name="ft")
                nc.sync.dma_start(out=ft[:], in_=feat9[:, pbase:pbase + Pc])
                ct = pool.tile([3, G], i32, tag="coors", name="ct")
                nc.sync.dma_start(out=ct[:], in_=coorsT[:, gbase:gbase + G])
                cent = pool.tile([3, G], f32, tag="cent", name="cent")
                nc.vector.tensor_copy(out=cent[:], in_=ct[:])
                nc.vector.tensor_scalar(
                    out=cent[:], in0=cent[:],
                    scalar1=wt["csb"][:, 0:1], scalar2=wt["csb"][:, 1:2],
                    op0=Alu.mult, op1=Alu.add)

                p0 = pool.tile([64, Pc], f32, tag="p0", name="p0")
                vf0 = pool.tile([64, G], f32, tag="vf0", name="vf0")
                p1 = pool.tile([128, Pc], f32, tag="p1", name="p1")
                vf1 = pool.tile([128, G], f32, tag="vf1", name="vf1")
                vh2a = pool.tile([128, G], f32, tag="vh2a", name="vh2a")
                vh2b = pool.tile([128, G], f32, tag="vh2b", name="vh2b")
                ot = pool.tile([16, G], f32, tag="ot", name="ot")

                def tiles():
                    for j in range(c):
                        t0 = 0
                        while t0 < G:
                            n = min(TILE_N, G - t0)
                            yield j, t0, n
                            t0 += n

                for j, t0, n in tiles():
                    cols = slice(j * G + t0, j * G + t0 + n)
                    gcol = slice(t0, t0 + n)
                    ps = pp.tile([64, n], f32, tag="h0", name="h0")
                    nc.tensor.matmul(ps[:], lhsT=r(wt["W0f"][:]), rhs=r(ft[:, cols]),
                                     start=True, stop=False)
                    nc.tensor.matmul(ps[:], lhsT=r(wt["W0c"][:]), rhs=r(cent[:, gcol]),
                                     start=False, stop=True)
                    nc.scalar.activation(p0[:, cols], ps[:], AF.Relu,
                                         bias=wt["ab0"][:, 1:2], scale=wt["ab0"][:, 0:1])
                    if j == 0:
                        nc.vector.tensor_copy(out=vf0[:, gcol], in_=p0[:, cols])
                    else:
                        nc.vector.tensor_max(out=vf0[:, gcol], in0=vf0[:, gcol],
                                             in1=p0[:, cols])
                for j, t0, n in tiles():
                    cols = slice(j * G + t0, j * G + t0 + n)
                    gcol = slice(t0, t0 + n)
                    ps = pp.tile([128, n], f32, tag="h1", name="h1")
                    nc.tensor.matmul(ps[:], lhsT=r(wt["W1a"][:]), rhs=r(p0[:, cols]),
                                     start=True, stop=False)
                    nc.tensor.matmul(ps[:], lhsT=r(wt["W1b"][:]), rhs=r(vf0[:, gcol]),
                                     start=False, stop=True)
                    nc.scalar.activation(p1[:, cols], ps[:], AF.Relu,
                                         bias=wt["ab1"][:, 1:2], scale=wt["ab1"][:, 0:1])
                    if j == 0:
                        nc.vector.tensor_copy(out=vf1[:, gcol], in_=p1[:, cols])
                    else:
                        nc.vector.tensor_max(out=vf1[:, gcol], in0=vf1[:, gcol],
                                             in1=p1[:, cols])
                for j, t0, n in tiles():
                    cols = slice(j * G + t0, j * G + t0 + n)
                    gcol = slice(t0, t0 + n)
                    psa = pp.tile([128, n], f32, tag="h2a", name="h2a")
                    psb = pp.tile([128, n], f32, tag="h2b", name="h2b")
                    nc.tensor.matmul(psa[:], lhsT=r(wt["W2aa"][:]), rhs=r(p1[:, cols]),
                                     start=True, stop=False)
                    nc.tensor.matmul(psa[:], lhsT=r(wt["W2ba"][:]), rhs=r(vf1[:, gcol]),
                                     start=False, stop=True)
                    nc.tensor.matmul(psb[:], lhsT=r(wt["W2ab"][:]), rhs=r(p1[:, cols]),
                                     start=True, stop=False)
                    nc.tensor.matmul(psb[:], lhsT=r(wt["W2bb"][:]), rhs=r(vf1[:, gcol]),
                                     start=False, stop=True)
                    if j == 0:
                        nc.vector.tensor_copy(out=vh2a[:, gcol], in_=psa[:])
                        nc.vector.tensor_copy(out=vh2b[:, gcol], in_=psb[:])
                    else:
                        nc.vector.tensor_max(out=vh2a[:, gcol], in0=vh2a[:, gcol], in1=psa[:])
                        nc.vector.tensor_max(out=vh2b[:, gcol], in0=vh2b[:, gcol], in1=psb[:])
                nc.scalar.activation(vh2a[:], vh2a[:], AF.Relu,
                                     bias=wt["ab2a"][:, 1:2], scale=wt["ab2a"][:, 0:1])
                nc.scalar.activation(vh2b[:], vh2b[:], AF.Relu,
                                     bias=wt["ab2b"][:, 1:2], scale=wt["ab2b"][:, 0:1])
                t0 = 0
                while t0 < G:
                    n = min(TILE_N, G - t0)
                    gcol = slice(t0, t0 + n)
                    ps = pp.tile([16, n], f32, tag="h0", name="hp")
                    nc.tensor.matmul(ps[:], lhsT=r(wt["Wpa"][:]), rhs=r(vh2a[:, gcol]),
                                     start=True, stop=False)
                    nc.tensor.matmul(ps[:], lhsT=r(wt["Wpb"][:]), rhs=r(vh2b[:, gcol]),
                                     start=False, stop=True)
                    nc.scalar.activation(ot[:, gcol], ps[:], AF.Relu, bias=wt["bp"][:, 0:1])
                    t0 += n
                nc.sync.dma_start(out=out_T[:, gbase:gbase + G], in_=ot[:])

                pbase += Pc
                gbase += G

    nc.compile()
    return nc


# --------------------------------------------------------------------------
# entry point
# --------------------------------------------------------------------------

def kernel(**inputs):
    from concourse.bass_utils import run_bass_kernel_spmd

    np_inputs = {k: np.asarray(v) for k, v in inputs.items()}
    inv = np_inputs["inv"].astype(np.int64)
    voxel_coors = np.asarray(np_inputs["voxel_coors"], np.int32)
    V = voxel_coors.shape[0]

    plan = build_plan(inv, V)
    in_maps = host_prepare(plan, np_inputs)
    nc = build_program(plan)
    res = run_bass_kernel_spmd(nc, in_maps, list(range(N_CORES)))
    core_outs = [np.asarray(res.results[d]["out_T"]) for d in range(N_CORES)]
    out = host_assemble(plan, core_outs, V)
    return out, voxel_coors
